# revision 78
# baseline (speedup 1.0000x reference)
"""Multi-head causal attention on 8 TRN2 NeuronCores (bf16, pipelined v3).

Sharding: core c -> (batch b = c//2, head-group g = c%2). Each core computes
Q/K/V projections for its 8 heads (512 of the 1024 channels), causal
attention, and the row-parallel W_o partial product; the host sums the two
partials per batch (the "all-reduce").

Attention per head h (d_k=64): scores computed transposed,
S^T = K_h @ Q_h^T (k on partitions, q on free axis), exp on the scalar
engine (no max subtraction: |scores/8| < ~6.5 at these scales),
multiplicative 0/1 mask on diagonal blocks only (one shared [128,2,512]
triangle tile works for every diagonal strip), and P^T feeds
out^T = [V_h | 1]^T @ P^T directly, whose row 64 accumulates the softmax
denominators Z. Head pairs run at partition offsets 0/64 so the two K=64
score matmuls occupy disjoint PE row-groups concurrently.

v3 scheduling changes vs the 312us v2 (now ~281us; PE streaming floor is
~196us, scalar-engine exp stream ~158us, input DMA ~240 GB/s achieved):
 - Attention steps run in PAIRS: both S matmul-pairs issue back-to-back so
   the scalar engine pipelines two exps per PE span (the 2-deep s2 psum
   ring holds exactly two in-flight score tiles).
 - tile_wait_until hints pin the Tile scheduler's DMA-arrival model to the
   measured queue throughput. Without them it front-loads matmuls whose x
   chunks arrive late, head-of-line-blocking the exp stream (~6us), and
   parks warm-up work where it is useless.
 - 44 chained warm-up matmuls with spread wait-hints blanket the ~24us
   input-DMA window; HAM now holds K=8/8 for the entire kernel (v2
   oscillated 8+ times costing ~22us of half-clock penalty).
 - No DMA ever issues from the scalar queue (DMA_DIRECT2D costs ~600ns of
   issuing-engine time and scalar paces the exp stream): inputs ride
   sync/gpsimd/scalar-only-before-first-exp, y writes alternate
   sync/gpsimd.
 - Pull rates {a0:3, a1:0, a2:1, a3:3}: a0 pre-pulls the x0-only V
   projections into the x1/x2/x3 arrival holes; a1/a2 are fed by their own
   ensure bursts. a3's LOW rate is critical: at 6 the wo(j-1) fillers were
   consumed in ~3 pairs and the last ~5 pairs of block (3,3) starved
   behind the exp pacer (HAM re-throttle, last exp at ~278us); at 3 they
   spread across the whole block and the exp stream ends ~6.5us earlier.
 - a=3 walks q-blocks ascending so wo(j) unlocks block-by-block and the
   scheduler hoists each wo chunk's c0-c2 accumulation ahead of the final
   normalize (only the 8 c3 matmuls wait on it).
 - Epilogue left-shift: the LAST block's y writes ride sync only, so
   gpsimd's final instruction is the last norm broadcast and its ~4.2us
   DSP-quiesce drain overlaps the y-write tail instead of serializing
   after it; the last block's yst casts and Z copies ride the post-exp-
   idle scalar engine, in parallel with the DVE norm muls; no tail
   dummies (wo runs warm anyway and they delayed the tensor teardown
   chain that gates the final barrier).
Remaining span (all verified pinned): ~8us TileContext barrier rounds,
~4us queue-bound input trickle (contiguous-block repack of x/y measured
neutral), ~17us AV LDWEIGHTS serialization (65-col loads are FWL/
background-buffer ineligible), ~3us s2-ring exp-latency stalls at a3
block boundaries (ps-ring bypass measured worse; vt tag-merge neutral).
"""

from collections import deque

import numpy as np

B, T, D = 4, 2048, 1024
NH, DK = 16, 64
NCORES = 8
HPC = NH // 2            # heads per core
HD = HPC * DK            # 512 head-dim channels per core
P = 128                  # partitions
NT = T // P              # 16 k-tiles
NQ = T // 512            # 4 q-blocks

_CACHE = {}


def _build():
    import concourse.mybir as mybir
    import concourse.tile as tile
    from concourse import bacc
    from concourse.tile import add_dep_helper

    f32, bf16 = mybir.dt.float32, mybir.dt.bfloat16
    Exp = mybir.ActivationFunctionType.Exp

    nc = bacc.Bacc(None, target_bir_lowering=False, debug=False)
    xT = nc.dram_tensor("xT", [D, T], bf16, kind="ExternalInput")
    wqT = nc.dram_tensor("wqT", [D, HD], bf16, kind="ExternalInput")
    wkT = nc.dram_tensor("wkT", [D, HD], bf16, kind="ExternalInput")
    wvT = nc.dram_tensor("wvT", [D, HD], bf16, kind="ExternalInput")
    woT = nc.dram_tensor("woT", [HD, D], bf16, kind="ExternalInput")
    mask = nc.dram_tensor("mask", [P, 2 * 512], bf16, kind="ExternalInput")
    yT = nc.dram_tensor("yT", [D, T], bf16, kind="ExternalOutput")

    with tile.TileContext(nc) as tc:
        with (
            tc.tile_pool(name="persist", bufs=1) as persist,
            tc.tile_pool(name="work", bufs=1) as work,
            tc.tile_pool(name="psum", bufs=1, space="PSUM") as psum,
        ):
            # ---- persistent tiles --------------------------------------
            xtc = [
                [persist.tile([P, 512], bf16, tag=f"x{c}_{t}",
                              name=f"x{c}_{t}")
                 for t in range(NQ)]
                for c in range(8)
            ]
            wq_sb = persist.tile([P, 8, HD], bf16, tag="wq")
            wk_sb = persist.tile([P, 8, HD], bf16, tag="wk")
            wv_sb = persist.tile([P, 8, HD], bf16, tag="wv")
            wo_sb = persist.tile([P, 4, D], bf16, tag="wo")
            mask_sb = persist.tile([P, 2, 512], bf16, tag="mask")
            qt = [persist.tile([P, T], bf16, tag=f"qt{a}", name=f"qt{a}")
                  for a in range(4)]
            kt = [persist.tile([P, T], bf16, tag=f"kt{a}", name=f"kt{a}")
                  for a in range(4)]
            vt = [persist.tile([P, HPC, DK + 1], bf16, tag=f"v{tt}",
                               name=f"v{tt}")
                  for tt in range(NT)]
            otn = [persist.tile([P, T], bf16, tag=f"otn{i}", name=f"otn{i}")
                   for i in range(4)]

            # ---- HAM warmup: dependency-free matmuls on garbage SBUF.
            # qt[3] is written much later, so reading it now costs nothing
            # (NaN results land in a psum bank that is overwritten with
            # start=True before any real use).
            def dummy_mms(n, tag, width=512, chain=False,
                          wait_base=None, wait_step=0.0):
                # chain=True: every matmul writes the same PSUM half, so
                # each waits the previous one's completion — a cheap
                # "activity blanket" that stretches n matmuls over a long
                # window with only ~50ns drain gaps (HAM never re-throttles)
                from contextlib import nullcontext
                wups = psum.tile([P, 2, 512], f32, tag="s2", bufs=2,
                                 name=f"wups_{tag}")
                for w in range(n):
                    ctx = (tc.tile_wait_until(wait_base + wait_step * w)
                           if wait_base is not None else nullcontext())
                    with ctx:
                        nc.tensor.matmul(
                            wups[:, 0 if chain else w % 2, 0:width],
                            lhsT=qt[3][0:P, 0:P],
                            rhs=qt[3][0:P, 512:512 + width],
                            start=True,
                            stop=True,
                        )

            # chained dummies with spread wait-hints: the scheduler
            # sprinkles them across the whole ~24us input-DMA window,
            # plugging the PE holes between chunk arrivals so HAM stays
            # at 8/8 until the dense stream takes over
            dummy_mms(44, "start", chain=True, wait_base=0.0055,
                      wait_step=0.00055)

            # ---- input DMAs: three parallel stages. Stage 1 is exactly
            # what attention block (0,0) needs (weights + mask + x tch0);
            # later x chunks are gated behind it so they don't steal HBM
            # bandwidth from the critical path. All DMAs issue from the
            # sync/gpsimd queues: DMA_DIRECT2D costs ~600ns of issuing-
            # engine time, and the scalar engine paces the exp stream.
            xT_r = xT.rearrange("(co p) t -> co p t", p=P)
            qrr = [nc.sync, nc.gpsimd]

            # stage 1 interleaved per contraction chunk: projection matmul
            # c can start as soon as (wq chunk c, x0 chunk c) land instead
            # of waiting for whole-tensor transfers
            wqT_r = wqT.rearrange("(co p) d -> co p d", p=P)
            wkT_r = wkT.rearrange("(co p) d -> co p d", p=P)
            wvT_r = wvT.rearrange("(co p) d -> co p d", p=P)
            # stage 1 across FOUR queues: scalar/vector are idle until the
            # first exp (~21us) / first psum cast (~13us), so their DMA
            # issue slots are free exactly when stage 1 runs
            # front-loaded: (wq, x0) pairs land first so the q projection
            # and the first S can start ~4us earlier; wk next (needed by
            # the same first S), wv last (first needed by AV two steps in)
            # tile_wait_until hints pin the scheduler's DMA-arrival model
            # to the ~240 GB/s reality — without them it front-loads
            # consumers of late chunks (x1-dependent projections) AHEAD of
            # the first S pair and head-of-line-blocks the exp stream.
            s1 = []
            s1.append(nc.sync.dma_start(
                out=mask_sb,
                in_=mask.rearrange("p (g q) -> p g q", q=512)).ins)
            wk_last = wv_last = None
            q3 = [nc.sync, nc.gpsimd, nc.scalar]
            for c in range(8):
                with tc.tile_wait_until(0.008 + 0.0015 * c):
                    s1.append(nc.sync.dma_start(
                        out=wq_sb[:, c, :], in_=wqT_r[c]).ins)
                    s1.append(nc.gpsimd.dma_start(
                        out=xtc[c][0], in_=xT_r[c][:, 0:512]).ins)
                    s1.append(nc.scalar.dma_start(
                        out=wk_sb[:, c, :], in_=wkT_r[c]).ins)
            wk_last = s1[-1]
            for c in range(8):
                with tc.tile_wait_until(0.017 + 0.0008 * c):
                    wv_last = q3[c % 3].dma_start(
                        out=wv_sb[:, c, :], in_=wvT_r[c]).ins
                s1.append(wv_last)
            x1last = None
            for c in range(8):
                with tc.tile_wait_until(0.024 + 0.0008 * c):
                    ins = qrr[c % 2].dma_start(
                        out=xtc[c][1], in_=xT_r[c][:, 512:1024])
                if c < 2:
                    add_dep_helper(ins.ins, wk_last, sync=True,
                                   reason="x tch1 after stage1 wk")
                    add_dep_helper(ins.ins, wv_last, sync=True,
                                   reason="x tch1 after stage1 wv")
                x1last = ins.ins
            for tch in (2, 3):
                for c in range(8):
                    with tc.tile_wait_until(0.0245 + 0.006 * tch
                                            + 0.0008 * c):
                        ins = qrr[c % 2].dma_start(
                            out=xtc[c][tch],
                            in_=xT_r[c][:, 512 * tch:512 * tch + 512])
                    add_dep_helper(ins.ins, x1last, sync=True,
                                   reason="x tail after tch1")
            with tc.tile_wait_until(0.045):
                ins = nc.gpsimd.dma_start(
                    out=wo_sb, in_=woT.rearrange("(co p) d -> p co d", p=P))
            add_dep_helper(ins.ins, x1last, sync=True, reason="wo late")
            for tt in range(NT):
                nc.vector.memset(vt[tt][:, :, DK:DK + 1], 1.0)

            # ---- filler machinery --------------------------------------
            fillq = deque()
            by_key = {}

            def add_ops(key, ops):
                for op in ops:
                    st = {"done": False}

                    def run(st=st, op=op):
                        if not st["done"]:
                            st["done"] = True
                            op()
                        else:
                            return True
                    run.st = st
                    fillq.append(run)
                    by_key.setdefault(key, []).append(run)

            def ensure(key):
                for run in by_key.get(key, ()):
                    run()

            def pull(n):
                while n > 0 and fillq:
                    run = fillq.popleft()
                    if run.st["done"]:
                        continue
                    run()
                    n -= 1

            def drain():
                while fillq:
                    run = fillq.popleft()
                    run()

            # ---- op builders (each closure emits one PE matmul) --------
            def v_tile_ops(tt):
                st = {}

                def mk(c):
                    def op():
                        if c == 0:
                            st["ps"] = psum.tile([P, HD], f32, tag="ps",
                                                 bufs=2, name=f"vps{tt}")
                        nc.tensor.matmul(
                            st["ps"],
                            lhsT=xtc[c][tt // 4][:, P * (tt % 4):P * (tt % 4) + P],
                            rhs=wv_sb[:, c, :],
                            start=(c == 0),
                            stop=(c == 7),
                        )
                        if c == 7:
                            nc.vector.tensor_copy(
                                vt[tt][:, :, 0:DK],
                                st["ps"].rearrange("p (h e) -> p h e", e=DK),
                            )
                    return op

                return [mk(c) for c in range(8)]

            def proj_tile_ops(nm, w_sb, out_sb, a, tch):
                st = {}

                def mk(c):
                    def op():
                        if c == 0:
                            st["ps"] = psum.tile([P, 512], f32, tag="ps",
                                                 bufs=2,
                                                 name=f"{nm}ps{a}_{tch}")
                        nc.tensor.matmul(
                            st["ps"],
                            lhsT=w_sb[:, c, 128 * a:128 * a + 128],
                            rhs=xtc[c][tch],
                            start=(c == 0),
                            stop=(c == 7),
                        )
                        if c == 7:
                            nc.vector.tensor_copy(
                                out_sb[:, 512 * tch:512 * tch + 512], st["ps"]
                            )
                    return op

                return [mk(c) for c in range(8)]

            def wo_tile_ops(dt_, tch):
                st = {}

                def mk(c):
                    def op():
                        if c == 0:
                            st["ps"] = psum.tile([P, 512], f32, tag="ps",
                                                 bufs=2,
                                                 name=f"yps{dt_}_{tch}")
                        nc.tensor.matmul(
                            st["ps"],
                            lhsT=wo_sb[:, c, 128 * dt_:128 * dt_ + 128],
                            rhs=otn[c][:, 512 * tch:512 * tch + 512],
                            start=(c == 0),
                            stop=(c == 3),
                        )
                        if c == 3:
                            yst = work.tile([P, 512], bf16, tag="yst", bufs=4,
                                            name=f"yst{dt_}_{tch}")
                            # the last block's casts split across the
                            # post-exp-idle scalar engine AND the DVE (its
                            # norm muls are done by then): 4+4 in parallel
                            # instead of 8 serializing ~4.5us on scalar
                            if tch == 3 and dt_ % 2 == 0:
                                nc.scalar.copy(yst, st["ps"])
                            else:
                                nc.vector.tensor_copy(yst, st["ps"])
                            # last block's y writes ride sync ONLY so
                            # gpsimd's ~4.2us DSP-quiesce epilogue drain
                            # (which runs after its final instruction)
                            # overlaps the sync y-write tail instead of
                            # serializing after it; earlier blocks keep
                            # the dual-queue split for transfer bandwidth
                            eng = (nc.sync if (tch == 3 or dt_ % 2 == 0)
                                   else nc.gpsimd)
                            eng.dma_start(
                                out=yT[128 * dt_:128 * dt_ + 128,
                                       512 * tch:512 * tch + 512],
                                in_=yst,
                            )
                    return op

                return [mk(c) for c in range(4)]

            # enqueue fillers in stream order. V tiles go AFTER the a=0
            # q/k projections: their wv-chunk DMAs land late in stage 1,
            # and pulled fillers that wait on DMA head-of-line-block the
            # in-order PE queue.
            for tch in range(NQ):
                add_ops(("q", 0, tch), proj_tile_ops("qt", wq_sb, qt[0], 0, tch))
                add_ops(("k", 0, tch), proj_tile_ops("kt", wk_sb, kt[0], 0, tch))
            for tt in range(NT):
                add_ops(("v", tt), v_tile_ops(tt))
            for a in range(1, 4):
                for tch in range(NQ):
                    add_ops(("q", a, tch),
                            proj_tile_ops("qt", wq_sb, qt[a], a, tch))
                for tch in range(NQ):
                    add_ops(("k", a, tch),
                            proj_tile_ops("kt", wk_sb, kt[a], a, tch))

            # ---- attention with one-k-tile-deferred AV -----------------
            pending = deque()

            def flush_pending(keep=0):
                while len(pending) > keep:
                    pending.popleft()()

            # a=0/a=1 are PE-bound through their own ensure chains — any
            # pull there steals the fillers that keep a=2/a=3 fed. a=2
            # pre-pulls q3/k3 slowly; a=3 eats the remnants plus the wo
            # ops that unlock block by block.
            # a=0 pulls feed the x0-only V projections into the holes
            # left by late x1/x2/x3 arrival; a=1 is PE-bound via its own
            # ensures; a=2 pre-pulls q3/k3 slowly. a=3's LOW rate spreads
            # the per-block wo unlocks across ALL pairs of the next block
            # instead of exhausting them in ~3 pairs and starving the
            # block tail behind the exp pacer.
            pull_rate = {0: 3, 1: 0, 2: 1, 3: 3}

            def ensure_block(a, j, with_v=True):
                if a >= 4 or j >= NQ:
                    return
                ensure(("q", a, j))
                for tch in range(j + 1):
                    ensure(("k", a, tch))
                if with_v:
                    for tt in range(4 * j + 4):
                        ensure(("v", tt))

            # a=3 also walks j ascending: wo(j) unlocks after norm(3,j)
            # and feeds the PE during blocks j+1.. — by block (3,3) the
            # q/k fillers are long gone and wo is the only filler left.
            for a in range(4):
                jorder = range(NQ)
                for j in jorder:
                    # V tiles are also ensured lazily inside the deferred
                    # AV ops (during a=0 they'd otherwise park 32
                    # DMA-blocked matmuls in front of the first score
                    # matmul)
                    ensure_block(a, j, with_v=(a >= 1))
                    avst = {}

                    def mk_av(a, j, k0, u_t, lo, first, last, avst):
                        def op():
                            ensure(("v", k0))
                            if first:
                                for hh in (0, 1):
                                    avst[hh] = psum.tile(
                                        [DK + 1, 512], f32, tag="av",
                                        bufs=2, name=f"av{a}_{j}_{hh}")
                            for hh in (0, 1):
                                nc.tensor.matmul(
                                    avst[hh][:, lo:512],
                                    lhsT=vt[k0][:, 2 * a + hh, :],
                                    rhs=u_t[:, hh, lo:512],
                                    start=first,
                                    stop=last,
                                )
                        return op

                    def mk_norm(a, j, avst):
                        # the last block's Z copies ride the then-idle
                        # scalar engine so both heads' chains overlap
                        last = (a == 3 and j == NQ - 1)

                        def op():
                            for hh in (0, 1):
                                poff = 64 * hh
                                z = work.tile([1, 512], f32, tag="z",
                                              bufs=2, name=f"z{a}_{j}")
                                if last:
                                    nc.scalar.copy(z, avst[hh][DK:DK + 1, :])
                                else:
                                    nc.vector.tensor_copy(
                                        z, avst[hh][DK:DK + 1, :])
                                rz = work.tile([1, 512], f32, tag="rz",
                                               bufs=2, name=f"rz{a}_{j}")
                                nc.vector.reciprocal_approx_fast(rz, z)
                                bc = work.tile([DK, 512], f32, tag="bc",
                                               bufs=2, name=f"bc{a}_{j}")
                                nc.gpsimd.partition_broadcast(bc, rz)
                                nc.vector.tensor_mul(
                                    otn[a][poff:poff + DK,
                                           512 * j:512 * j + 512],
                                    avst[hh][0:DK, :],
                                    bc,
                                )
                        return op

                    nk = 4 * j + 4
                    # steps run in PAIRS: both S matmul pairs issue
                    # back-to-back so the scalar engine pipelines two exps
                    # per PE span (the s2 psum ring holds exactly 2). In
                    # PE-bound phases this keeps the exp stream compressed
                    # instead of one-exp-per-filler-span.
                    for k0 in range(0, nk, 2):
                        us = []
                        for k in (k0, k0 + 1):
                            r = k - 4 * j
                            lo = 128 * r if r > 0 else 0
                            s_ps = psum.tile([P, 2, 512], f32, tag="s2",
                                             bufs=2, name=f"sps{a}_{j}_{k}")
                            for hh in (0, 1):
                                poff = 64 * hh
                                nc.tensor.matmul(
                                    s_ps[:, hh, lo:512],
                                    lhsT=kt[a][poff:poff + 64,
                                               P * k:P * k + P],
                                    rhs=qt[a][poff:poff + 64,
                                              512 * j + lo:512 * j + 512],
                                    start=True,
                                    stop=True,
                                )
                            u_t = work.tile([P, 2, 512], bf16, tag="u",
                                            bufs=6, name=f"u{a}_{j}_{k}")
                            nc.scalar.activation(
                                u_t[:, :, lo:512], s_ps[:, :, lo:512], Exp,
                                scale=0.125,
                            )
                            if r >= 0:
                                nc.vector.tensor_mul(
                                    u_t[:, :, lo:512],
                                    u_t[:, :, lo:512],
                                    mask_sb[:, :, 0:512 - lo],
                                )
                            us.append((k, u_t, lo))
                        # k0==0: flush everything incl. the previous
                        # block's normalize; keep=2 afterwards defers each
                        # pair's AVs past the next pair's S matmuls
                        flush_pending(keep=0 if k0 == 0 else 2)
                        pull(pull_rate[a] * 2)
                        for k, u_t, lo in us:
                            pending.append(
                                mk_av(a, j, k, u_t, lo, k == 0, k == nk - 1,
                                      avst))
                    pending.append(mk_norm(a, j, avst))
                    if a == 3:
                        for dt_ in range(8):
                            add_ops(("wo", j), wo_tile_ops(dt_, j))

            flush_pending()
            # a few tail dummies cover the ~3.6us final-norm gap (the
            # rate_3 fix moved the whole tail earlier, so the last wo
            # group now sits right behind the norm chain and was getting
            # HAM-re-throttled); they fill genuine PE idle, not the
            # teardown path
            dummy_mms(14, "tail")
            drain()

    nc.finalize()
    return nc


def _get_nc():
    if "nc" not in _CACHE:
        _CACHE["nc"] = _build()
    return _CACHE["nc"]


def kernel(x, W_q, W_k, W_v, W_o):
    import ml_dtypes
    from concourse.bass_utils import run_bass_kernel_spmd

    bf16 = ml_dtypes.bfloat16
    x = np.asarray(x, dtype=np.float32)
    W_q = np.asarray(W_q, dtype=np.float32)
    W_k = np.asarray(W_k, dtype=np.float32)
    W_v = np.asarray(W_v, dtype=np.float32)
    W_o = np.asarray(W_o, dtype=np.float32)

    kk = np.arange(P)[:, None]
    uu = np.arange(512)[None, :]
    mask = np.tile((uu >= kk), (1, 2)).astype(bf16)

    in_maps = []
    for c in range(NCORES):
        b, g = c // 2, c % 2
        rows = slice(HD * g, HD * g + HD)
        in_maps.append(
            {
                "xT": np.ascontiguousarray(x[b].T).astype(bf16),
                "wqT": np.ascontiguousarray(W_q[rows, :].T).astype(bf16),
                "wkT": np.ascontiguousarray(W_k[rows, :].T).astype(bf16),
                "wvT": np.ascontiguousarray(W_v[rows, :].T).astype(bf16),
                "woT": np.ascontiguousarray(W_o[:, rows].T).astype(bf16),
                "mask": mask,
            }
        )

    res = run_bass_kernel_spmd(_get_nc(), in_maps, list(range(NCORES)))
    y = np.zeros((B, T, D), np.float32)
    for c in range(NCORES):
        y[c // 2] += res.results[c]["yT"].T.astype(np.float32)
    return y



# revision 79
# speedup vs baseline: 1.0019x; 1.0019x over previous
"""Multi-head causal attention on 8 TRN2 NeuronCores (bf16, pipelined v3).

Sharding: core c -> (batch b = c//2, head-group g = c%2). Each core computes
Q/K/V projections for its 8 heads (512 of the 1024 channels), causal
attention, and the row-parallel W_o partial product; the host sums the two
partials per batch (the "all-reduce").

Attention per head h (d_k=64): scores computed transposed,
S^T = K_h @ Q_h^T (k on partitions, q on free axis), exp on the scalar
engine (no max subtraction: |scores/8| < ~6.5 at these scales),
multiplicative 0/1 mask on diagonal blocks only (one shared [128,2,512]
triangle tile works for every diagonal strip), and P^T feeds
out^T = [V_h | 1]^T @ P^T directly, whose row 64 accumulates the softmax
denominators Z. Head pairs run at partition offsets 0/64 so the two K=64
score matmuls occupy disjoint PE row-groups concurrently.

v3 scheduling changes vs the 312us v2 (now ~281us; PE streaming floor is
~196us, scalar-engine exp stream ~158us, input DMA ~240 GB/s achieved):
 - Attention steps run in PAIRS: both S matmul-pairs issue back-to-back so
   the scalar engine pipelines two exps per PE span (the 2-deep s2 psum
   ring holds exactly two in-flight score tiles).
 - tile_wait_until hints pin the Tile scheduler's DMA-arrival model to the
   measured queue throughput. Without them it front-loads matmuls whose x
   chunks arrive late, head-of-line-blocking the exp stream (~6us), and
   parks warm-up work where it is useless.
 - 44 chained warm-up matmuls with spread wait-hints blanket the ~24us
   input-DMA window; HAM now holds K=8/8 for the entire kernel (v2
   oscillated 8+ times costing ~22us of half-clock penalty).
 - No DMA ever issues from the scalar queue (DMA_DIRECT2D costs ~600ns of
   issuing-engine time and scalar paces the exp stream): inputs ride
   sync/gpsimd/scalar-only-before-first-exp, y writes alternate
   sync/gpsimd.
 - Pull rates {a0:3, a1:0, a2:1, a3:3}: a0 pre-pulls the x0-only V
   projections into the x1/x2/x3 arrival holes; a1/a2 are fed by their own
   ensure bursts. a3's LOW rate is critical: at 6 the wo(j-1) fillers were
   consumed in ~3 pairs and the last ~5 pairs of block (3,3) starved
   behind the exp pacer (HAM re-throttle, last exp at ~278us); at 3 they
   spread across the whole block and the exp stream ends ~6.5us earlier.
 - a=3 walks q-blocks ascending so wo(j) unlocks block-by-block and the
   scheduler hoists each wo chunk's c0-c2 accumulation ahead of the final
   normalize (only the 8 c3 matmuls wait on it).
 - Epilogue left-shift: the LAST block's y writes ride sync only, so
   gpsimd's final instruction is the last norm broadcast and its ~4.2us
   DSP-quiesce drain overlaps the y-write tail instead of serializing
   after it; the last block's yst casts and Z copies ride the post-exp-
   idle scalar engine, in parallel with the DVE norm muls; 8 tail
   dummies cover the ~3.6us final-norm gap so the closing wo c3 matmuls
   are not HAM-re-throttled (needed again after the rate_3 fix moved
   the whole tail earlier).
Remaining span (all verified pinned): ~8us TileContext barrier rounds,
~4us queue-bound input trickle (contiguous-block repack of x/y measured
neutral), ~17us AV LDWEIGHTS serialization (65-col loads are FWL/
background-buffer ineligible), ~3us s2-ring exp-latency stalls at a3
block boundaries (ps-ring bypass measured worse; vt tag-merge neutral).
"""

from collections import deque

import numpy as np

B, T, D = 4, 2048, 1024
NH, DK = 16, 64
NCORES = 8
HPC = NH // 2            # heads per core
HD = HPC * DK            # 512 head-dim channels per core
P = 128                  # partitions
NT = T // P              # 16 k-tiles
NQ = T // 512            # 4 q-blocks

_CACHE = {}


def _build():
    import concourse.mybir as mybir
    import concourse.tile as tile
    from concourse import bacc
    from concourse.tile import add_dep_helper

    f32, bf16 = mybir.dt.float32, mybir.dt.bfloat16
    Exp = mybir.ActivationFunctionType.Exp

    nc = bacc.Bacc(None, target_bir_lowering=False, debug=False)
    xT = nc.dram_tensor("xT", [D, T], bf16, kind="ExternalInput")
    wqT = nc.dram_tensor("wqT", [D, HD], bf16, kind="ExternalInput")
    wkT = nc.dram_tensor("wkT", [D, HD], bf16, kind="ExternalInput")
    wvT = nc.dram_tensor("wvT", [D, HD], bf16, kind="ExternalInput")
    woT = nc.dram_tensor("woT", [HD, D], bf16, kind="ExternalInput")
    mask = nc.dram_tensor("mask", [P, 2 * 512], bf16, kind="ExternalInput")
    yT = nc.dram_tensor("yT", [D, T], bf16, kind="ExternalOutput")

    with tile.TileContext(nc) as tc:
        with (
            tc.tile_pool(name="persist", bufs=1) as persist,
            tc.tile_pool(name="work", bufs=1) as work,
            tc.tile_pool(name="psum", bufs=1, space="PSUM") as psum,
        ):
            # ---- persistent tiles --------------------------------------
            xtc = [
                [persist.tile([P, 512], bf16, tag=f"x{c}_{t}",
                              name=f"x{c}_{t}")
                 for t in range(NQ)]
                for c in range(8)
            ]
            wq_sb = persist.tile([P, 8, HD], bf16, tag="wq")
            wk_sb = persist.tile([P, 8, HD], bf16, tag="wk")
            wv_sb = persist.tile([P, 8, HD], bf16, tag="wv")
            wo_sb = persist.tile([P, 4, D], bf16, tag="wo")
            mask_sb = persist.tile([P, 2, 512], bf16, tag="mask")
            qt = [persist.tile([P, T], bf16, tag=f"qt{a}", name=f"qt{a}")
                  for a in range(4)]
            kt = [persist.tile([P, T], bf16, tag=f"kt{a}", name=f"kt{a}")
                  for a in range(4)]
            vt = [persist.tile([P, HPC, DK + 1], bf16, tag=f"v{tt}",
                               name=f"v{tt}")
                  for tt in range(NT)]
            otn = [persist.tile([P, T], bf16, tag=f"otn{i}", name=f"otn{i}")
                   for i in range(4)]

            # ---- HAM warmup: dependency-free matmuls on garbage SBUF.
            # qt[3] is written much later, so reading it now costs nothing
            # (NaN results land in a psum bank that is overwritten with
            # start=True before any real use).
            def dummy_mms(n, tag, width=512, chain=False,
                          wait_base=None, wait_step=0.0):
                # chain=True: every matmul writes the same PSUM half, so
                # each waits the previous one's completion — a cheap
                # "activity blanket" that stretches n matmuls over a long
                # window with only ~50ns drain gaps (HAM never re-throttles)
                from contextlib import nullcontext
                wups = psum.tile([P, 2, 512], f32, tag="s2", bufs=2,
                                 name=f"wups_{tag}")
                for w in range(n):
                    ctx = (tc.tile_wait_until(wait_base + wait_step * w)
                           if wait_base is not None else nullcontext())
                    with ctx:
                        nc.tensor.matmul(
                            wups[:, 0 if chain else w % 2, 0:width],
                            lhsT=qt[3][0:P, 0:P],
                            rhs=qt[3][0:P, 512:512 + width],
                            start=True,
                            stop=True,
                        )

            # chained dummies with spread wait-hints: the scheduler
            # sprinkles them across the whole ~24us input-DMA window,
            # plugging the PE holes between chunk arrivals so HAM stays
            # at 8/8 until the dense stream takes over
            dummy_mms(44, "start", chain=True, wait_base=0.0055,
                      wait_step=0.00055)

            # ---- input DMAs: three parallel stages. Stage 1 is exactly
            # what attention block (0,0) needs (weights + mask + x tch0);
            # later x chunks are gated behind it so they don't steal HBM
            # bandwidth from the critical path. All DMAs issue from the
            # sync/gpsimd queues: DMA_DIRECT2D costs ~600ns of issuing-
            # engine time, and the scalar engine paces the exp stream.
            xT_r = xT.rearrange("(co p) t -> co p t", p=P)
            qrr = [nc.sync, nc.gpsimd]

            # stage 1 interleaved per contraction chunk: projection matmul
            # c can start as soon as (wq chunk c, x0 chunk c) land instead
            # of waiting for whole-tensor transfers
            wqT_r = wqT.rearrange("(co p) d -> co p d", p=P)
            wkT_r = wkT.rearrange("(co p) d -> co p d", p=P)
            wvT_r = wvT.rearrange("(co p) d -> co p d", p=P)
            # stage 1 across FOUR queues: scalar/vector are idle until the
            # first exp (~21us) / first psum cast (~13us), so their DMA
            # issue slots are free exactly when stage 1 runs
            # front-loaded: (wq, x0) pairs land first so the q projection
            # and the first S can start ~4us earlier; wk next (needed by
            # the same first S), wv last (first needed by AV two steps in)
            # tile_wait_until hints pin the scheduler's DMA-arrival model
            # to the ~240 GB/s reality — without them it front-loads
            # consumers of late chunks (x1-dependent projections) AHEAD of
            # the first S pair and head-of-line-blocks the exp stream.
            s1 = []
            s1.append(nc.sync.dma_start(
                out=mask_sb,
                in_=mask.rearrange("p (g q) -> p g q", q=512)).ins)
            wk_last = wv_last = None
            q3 = [nc.sync, nc.gpsimd, nc.scalar]
            for c in range(8):
                with tc.tile_wait_until(0.008 + 0.0015 * c):
                    s1.append(nc.sync.dma_start(
                        out=wq_sb[:, c, :], in_=wqT_r[c]).ins)
                    s1.append(nc.gpsimd.dma_start(
                        out=xtc[c][0], in_=xT_r[c][:, 0:512]).ins)
                    s1.append(nc.scalar.dma_start(
                        out=wk_sb[:, c, :], in_=wkT_r[c]).ins)
            wk_last = s1[-1]
            for c in range(8):
                with tc.tile_wait_until(0.017 + 0.0008 * c):
                    wv_last = q3[c % 3].dma_start(
                        out=wv_sb[:, c, :], in_=wvT_r[c]).ins
                s1.append(wv_last)
            x1last = None
            for c in range(8):
                with tc.tile_wait_until(0.024 + 0.0008 * c):
                    ins = qrr[c % 2].dma_start(
                        out=xtc[c][1], in_=xT_r[c][:, 512:1024])
                if c < 2:
                    add_dep_helper(ins.ins, wk_last, sync=True,
                                   reason="x tch1 after stage1 wk")
                    add_dep_helper(ins.ins, wv_last, sync=True,
                                   reason="x tch1 after stage1 wv")
                x1last = ins.ins
            for tch in (2, 3):
                for c in range(8):
                    with tc.tile_wait_until(0.0245 + 0.006 * tch
                                            + 0.0008 * c):
                        ins = qrr[c % 2].dma_start(
                            out=xtc[c][tch],
                            in_=xT_r[c][:, 512 * tch:512 * tch + 512])
                    add_dep_helper(ins.ins, x1last, sync=True,
                                   reason="x tail after tch1")
            with tc.tile_wait_until(0.045):
                ins = nc.gpsimd.dma_start(
                    out=wo_sb, in_=woT.rearrange("(co p) d -> p co d", p=P))
            add_dep_helper(ins.ins, x1last, sync=True, reason="wo late")
            for tt in range(NT):
                nc.vector.memset(vt[tt][:, :, DK:DK + 1], 1.0)

            # ---- filler machinery --------------------------------------
            fillq = deque()
            by_key = {}

            def add_ops(key, ops):
                for op in ops:
                    st = {"done": False}

                    def run(st=st, op=op):
                        if not st["done"]:
                            st["done"] = True
                            op()
                        else:
                            return True
                    run.st = st
                    fillq.append(run)
                    by_key.setdefault(key, []).append(run)

            def ensure(key):
                for run in by_key.get(key, ()):
                    run()

            def pull(n):
                while n > 0 and fillq:
                    run = fillq.popleft()
                    if run.st["done"]:
                        continue
                    run()
                    n -= 1

            def drain():
                while fillq:
                    run = fillq.popleft()
                    run()

            # ---- op builders (each closure emits one PE matmul) --------
            def v_tile_ops(tt):
                st = {}

                def mk(c):
                    def op():
                        if c == 0:
                            st["ps"] = psum.tile([P, HD], f32, tag="ps",
                                                 bufs=2, name=f"vps{tt}")
                        nc.tensor.matmul(
                            st["ps"],
                            lhsT=xtc[c][tt // 4][:, P * (tt % 4):P * (tt % 4) + P],
                            rhs=wv_sb[:, c, :],
                            start=(c == 0),
                            stop=(c == 7),
                        )
                        if c == 7:
                            nc.vector.tensor_copy(
                                vt[tt][:, :, 0:DK],
                                st["ps"].rearrange("p (h e) -> p h e", e=DK),
                            )
                    return op

                return [mk(c) for c in range(8)]

            def proj_tile_ops(nm, w_sb, out_sb, a, tch):
                st = {}

                def mk(c):
                    def op():
                        if c == 0:
                            st["ps"] = psum.tile([P, 512], f32, tag="ps",
                                                 bufs=2,
                                                 name=f"{nm}ps{a}_{tch}")
                        nc.tensor.matmul(
                            st["ps"],
                            lhsT=w_sb[:, c, 128 * a:128 * a + 128],
                            rhs=xtc[c][tch],
                            start=(c == 0),
                            stop=(c == 7),
                        )
                        if c == 7:
                            nc.vector.tensor_copy(
                                out_sb[:, 512 * tch:512 * tch + 512], st["ps"]
                            )
                    return op

                return [mk(c) for c in range(8)]

            def wo_tile_ops(dt_, tch):
                st = {}

                def mk(c):
                    def op():
                        if c == 0:
                            st["ps"] = psum.tile([P, 512], f32, tag="ps",
                                                 bufs=2,
                                                 name=f"yps{dt_}_{tch}")
                        nc.tensor.matmul(
                            st["ps"],
                            lhsT=wo_sb[:, c, 128 * dt_:128 * dt_ + 128],
                            rhs=otn[c][:, 512 * tch:512 * tch + 512],
                            start=(c == 0),
                            stop=(c == 3),
                        )
                        if c == 3:
                            yst = work.tile([P, 512], bf16, tag="yst", bufs=4,
                                            name=f"yst{dt_}_{tch}")
                            # the last block's casts split across the
                            # post-exp-idle scalar engine AND the DVE (its
                            # norm muls are done by then): 4+4 in parallel
                            # instead of 8 serializing ~4.5us on scalar
                            if tch == 3 and dt_ % 2 == 0:
                                nc.scalar.copy(yst, st["ps"])
                            else:
                                nc.vector.tensor_copy(yst, st["ps"])
                            # last block's y writes ride sync ONLY so
                            # gpsimd's ~4.2us DSP-quiesce epilogue drain
                            # (which runs after its final instruction)
                            # overlaps the sync y-write tail instead of
                            # serializing after it; earlier blocks keep
                            # the dual-queue split for transfer bandwidth
                            eng = (nc.sync if (tch == 3 or dt_ % 2 == 0)
                                   else nc.gpsimd)
                            eng.dma_start(
                                out=yT[128 * dt_:128 * dt_ + 128,
                                       512 * tch:512 * tch + 512],
                                in_=yst,
                            )
                    return op

                return [mk(c) for c in range(4)]

            # enqueue fillers in stream order. V tiles go AFTER the a=0
            # q/k projections: their wv-chunk DMAs land late in stage 1,
            # and pulled fillers that wait on DMA head-of-line-block the
            # in-order PE queue.
            for tch in range(NQ):
                add_ops(("q", 0, tch), proj_tile_ops("qt", wq_sb, qt[0], 0, tch))
                add_ops(("k", 0, tch), proj_tile_ops("kt", wk_sb, kt[0], 0, tch))
            for tt in range(NT):
                add_ops(("v", tt), v_tile_ops(tt))
            for a in range(1, 4):
                for tch in range(NQ):
                    add_ops(("q", a, tch),
                            proj_tile_ops("qt", wq_sb, qt[a], a, tch))
                for tch in range(NQ):
                    add_ops(("k", a, tch),
                            proj_tile_ops("kt", wk_sb, kt[a], a, tch))

            # ---- attention with one-k-tile-deferred AV -----------------
            pending = deque()

            def flush_pending(keep=0):
                while len(pending) > keep:
                    pending.popleft()()

            # a=0/a=1 are PE-bound through their own ensure chains — any
            # pull there steals the fillers that keep a=2/a=3 fed. a=2
            # pre-pulls q3/k3 slowly; a=3 eats the remnants plus the wo
            # ops that unlock block by block.
            # a=0 pulls feed the x0-only V projections into the holes
            # left by late x1/x2/x3 arrival; a=1 is PE-bound via its own
            # ensures; a=2 pre-pulls q3/k3 slowly. a=3's LOW rate spreads
            # the per-block wo unlocks across ALL pairs of the next block
            # instead of exhausting them in ~3 pairs and starving the
            # block tail behind the exp pacer.
            pull_rate = {0: 3, 1: 0, 2: 1, 3: 3}

            def ensure_block(a, j, with_v=True):
                if a >= 4 or j >= NQ:
                    return
                ensure(("q", a, j))
                for tch in range(j + 1):
                    ensure(("k", a, tch))
                if with_v:
                    for tt in range(4 * j + 4):
                        ensure(("v", tt))

            # a=3 also walks j ascending: wo(j) unlocks after norm(3,j)
            # and feeds the PE during blocks j+1.. — by block (3,3) the
            # q/k fillers are long gone and wo is the only filler left.
            for a in range(4):
                jorder = range(NQ)
                for j in jorder:
                    # V tiles are also ensured lazily inside the deferred
                    # AV ops (during a=0 they'd otherwise park 32
                    # DMA-blocked matmuls in front of the first score
                    # matmul)
                    ensure_block(a, j, with_v=(a >= 1))
                    avst = {}

                    def mk_av(a, j, k0, u_t, lo, first, last, avst):
                        def op():
                            ensure(("v", k0))
                            if first:
                                for hh in (0, 1):
                                    avst[hh] = psum.tile(
                                        [DK + 1, 512], f32, tag="av",
                                        bufs=2, name=f"av{a}_{j}_{hh}")
                            for hh in (0, 1):
                                nc.tensor.matmul(
                                    avst[hh][:, lo:512],
                                    lhsT=vt[k0][:, 2 * a + hh, :],
                                    rhs=u_t[:, hh, lo:512],
                                    start=first,
                                    stop=last,
                                )
                        return op

                    def mk_norm(a, j, avst):
                        # the last block's Z copies ride the then-idle
                        # scalar engine so both heads' chains overlap
                        last = (a == 3 and j == NQ - 1)

                        def op():
                            for hh in (0, 1):
                                poff = 64 * hh
                                z = work.tile([1, 512], f32, tag="z",
                                              bufs=2, name=f"z{a}_{j}")
                                if last:
                                    nc.scalar.copy(z, avst[hh][DK:DK + 1, :])
                                else:
                                    nc.vector.tensor_copy(
                                        z, avst[hh][DK:DK + 1, :])
                                rz = work.tile([1, 512], f32, tag="rz",
                                               bufs=2, name=f"rz{a}_{j}")
                                nc.vector.reciprocal_approx_fast(rz, z)
                                bc = work.tile([DK, 512], f32, tag="bc",
                                               bufs=2, name=f"bc{a}_{j}")
                                nc.gpsimd.partition_broadcast(bc, rz)
                                nc.vector.tensor_mul(
                                    otn[a][poff:poff + DK,
                                           512 * j:512 * j + 512],
                                    avst[hh][0:DK, :],
                                    bc,
                                )
                        return op

                    nk = 4 * j + 4
                    # steps run in PAIRS: both S matmul pairs issue
                    # back-to-back so the scalar engine pipelines two exps
                    # per PE span (the s2 psum ring holds exactly 2). In
                    # PE-bound phases this keeps the exp stream compressed
                    # instead of one-exp-per-filler-span.
                    for k0 in range(0, nk, 2):
                        us = []
                        for k in (k0, k0 + 1):
                            r = k - 4 * j
                            lo = 128 * r if r > 0 else 0
                            s_ps = psum.tile([P, 2, 512], f32, tag="s2",
                                             bufs=2, name=f"sps{a}_{j}_{k}")
                            for hh in (0, 1):
                                poff = 64 * hh
                                nc.tensor.matmul(
                                    s_ps[:, hh, lo:512],
                                    lhsT=kt[a][poff:poff + 64,
                                               P * k:P * k + P],
                                    rhs=qt[a][poff:poff + 64,
                                              512 * j + lo:512 * j + 512],
                                    start=True,
                                    stop=True,
                                )
                            u_t = work.tile([P, 2, 512], bf16, tag="u",
                                            bufs=6, name=f"u{a}_{j}_{k}")
                            nc.scalar.activation(
                                u_t[:, :, lo:512], s_ps[:, :, lo:512], Exp,
                                scale=0.125,
                            )
                            if r >= 0:
                                nc.vector.tensor_mul(
                                    u_t[:, :, lo:512],
                                    u_t[:, :, lo:512],
                                    mask_sb[:, :, 0:512 - lo],
                                )
                            us.append((k, u_t, lo))
                        # k0==0: flush everything incl. the previous
                        # block's normalize; keep=2 afterwards defers each
                        # pair's AVs past the next pair's S matmuls
                        flush_pending(keep=0 if k0 == 0 else 2)
                        pull(pull_rate[a] * 2)
                        for k, u_t, lo in us:
                            pending.append(
                                mk_av(a, j, k, u_t, lo, k == 0, k == nk - 1,
                                      avst))
                    pending.append(mk_norm(a, j, avst))
                    if a == 3:
                        for dt_ in range(8):
                            add_ops(("wo", j), wo_tile_ops(dt_, j))

            flush_pending()
            # a few tail dummies cover the ~3.6us final-norm gap (the
            # rate_3 fix moved the whole tail earlier, so the last wo
            # group now sits right behind the norm chain and was getting
            # HAM-re-throttled); they fill genuine PE idle, not the
            # teardown path
            dummy_mms(8, "tail")
            drain()

    nc.finalize()
    return nc


def _get_nc():
    if "nc" not in _CACHE:
        _CACHE["nc"] = _build()
    return _CACHE["nc"]


def kernel(x, W_q, W_k, W_v, W_o):
    import ml_dtypes
    from concourse.bass_utils import run_bass_kernel_spmd

    bf16 = ml_dtypes.bfloat16
    x = np.asarray(x, dtype=np.float32)
    W_q = np.asarray(W_q, dtype=np.float32)
    W_k = np.asarray(W_k, dtype=np.float32)
    W_v = np.asarray(W_v, dtype=np.float32)
    W_o = np.asarray(W_o, dtype=np.float32)

    kk = np.arange(P)[:, None]
    uu = np.arange(512)[None, :]
    mask = np.tile((uu >= kk), (1, 2)).astype(bf16)

    in_maps = []
    for c in range(NCORES):
        b, g = c // 2, c % 2
        rows = slice(HD * g, HD * g + HD)
        in_maps.append(
            {
                "xT": np.ascontiguousarray(x[b].T).astype(bf16),
                "wqT": np.ascontiguousarray(W_q[rows, :].T).astype(bf16),
                "wkT": np.ascontiguousarray(W_k[rows, :].T).astype(bf16),
                "wvT": np.ascontiguousarray(W_v[rows, :].T).astype(bf16),
                "woT": np.ascontiguousarray(W_o[:, rows].T).astype(bf16),
                "mask": mask,
            }
        )

    res = run_bass_kernel_spmd(_get_nc(), in_maps, list(range(NCORES)))
    y = np.zeros((B, T, D), np.float32)
    for c in range(NCORES):
        y[c // 2] += res.results[c]["yT"].T.astype(np.float32)
    return y

